# revision 26
# baseline (speedup 1.0000x reference)
"""Trainium2 Bass kernel for nn_CNNInteractLayer (CNN interaction layer).

Math: for each episode b, s-row i, q-row j:
  out[b,i,j] = maxpool_L(relu(conv_k(concat(s[b,i], q[b,j])))) for k in 2..5
Factorization: conv(concat(s,q)) = conv_s(s) + conv_q(q) + bias, so per-row
convolutions are computed once (25 s-rows + 13 q-rows per core) on the PE
(bf16, tap-shifts accumulated in PSUM). The pairwise combine runs entirely
on the vector engines in bf16: broadcast adds (stride-0 APs) form
sum[ch, i, t, l] on DVE, then the relu'd max over the L=31 window is a
tensor_tensor max tree. Work splits cleanly: the add over s-rows 0..12
(pairs 0..169) only needs the first s conv chunk and its max tree runs on
GPSIMD, while DVE trees the s-rows 13..24 half it just added — so DVE and
GPSIMD never wait on each other's output.

No DRAM staging roundtrip and no pairwise matmul: the PE only does the conv,
and PE / DVE / GPSIMD run ~balanced (~8us per channel chunk each).

Sharding: 8 cores = 4 episodes x 2 halves of the q-row range.
"""

import os
import sys

import numpy as np

for _p in ("/opt/trn_rl_repo",):
    if os.path.isdir(_p) and _p not in sys.path:
        sys.path.insert(0, _p)

# the bass runner needs the axon jax backend; don't let a cpu-only pin hide it
if "axon" not in os.environ.get("JAX_PLATFORMS", "axon"):
    os.environ.pop("JAX_PLATFORMS", None)

import ml_dtypes  # noqa: E402

from concourse import bacc, bass, mybir, tile  # noqa: E402
from concourse.bass_utils import run_bass_kernel_spmd  # noqa: E402

# Problem dims (hardcoded per spec)
B, N, K, Q, L, D = 4, 5, 5, 5, 31, 512
NROW = N * K            # 25 s-rows per episode
NQROW = N * Q           # 25 q-rows per episode
JN = 13                 # q-rows per core (odd cores have 12 real + 1 dup)
NPAIR = NROW * JN       # 325 pairs per core
ROWSTR = L + 4          # 35: 2-col halo each side per row
PS_COLS = NROW * ROWSTR  # 875
PQ_COLS = JN * ROWSTR    # 455
NCH = 600               # device channels: [k5 | k4 | k3 | k2] x 150
NCK = 5                 # channel chunks
CCW = 120               # channels per chunk
DELTAS = [(-2, 300), (-1, 600), (0, 600), (1, 450), (2, 150)]
# per-chunk emit order: delta -1 first and 0 last are full width (start/stop
# matmuls of a PSUM accumulation group must cover the full partition range)
DELTA_ORDER = [1, 0, 3, 4, 2]
PAD_OF_K = {2: 1, 3: 1, 4: 2, 5: 2}
ORD_OF_K = {5: 0, 4: 1, 3: 2, 2: 3}
SRA = 13                # s-rows in the first conv/add block (A); rest in B
JNB = JN - 1            # block B q-window: dev_t 1..13 (12 q-rows)
NPA = SRA * JN          # 169 pairs in block A
NPB = (NROW - SRA) * JNB  # 144 pairs in block B
NPDEV = NPA + NPB       # 313 device pairs per core (625 split 313/312)
# GPSIMD (Pool) can only ADD (its ucode has no tensor_tensor max), at 0.42
# efficiency. It adds the first POOLR[ci] s-rows' pairs of each chunk; DVE
# adds the rest and runs every max tree. Pool's slice of chunk ci is treed
# one chunk later so DVE never waits on it. First/last chunks give Pool
# less so the pipeline head/tail stay DVE-dense.
POOLR = [5, 11, 11, 11, 11]  # s-rows added by Pool, per processed chunk
# conv position chunks, in emit order: q first (its DMA lands first), then
# the two s blocks
POS_CHUNKS = [(1, 0, JN), (0, 0, SRA), (0, SRA, NROW - SRA)]
WSIDE = 2100            # (channel, tap) pairs per side
NWARM = 20              # PE p-state warmup matmuls (overlap input DMA)
CHUNK_ORDER = [4, 3, 2, 1, 0]  # small chunks first: shorter pipeline head


def _piece_tables():
    """Per chunk: valid delta pieces and their packed-W column offsets.

    Packed W layout [D, 2*WSIDE], chunk-major: for each chunk, side-s block
    then side-q block; within a side the valid delta pieces in DELTA_ORDER.
    """
    pieces = []   # pieces[cc] = [(di, delta, width), ...] in emit order
    wcol = {}     # (cc, side, di) -> packed column
    choff = [0]
    off = 0
    for cc in range(NCK):
        c0 = cc * CCW
        ps = []
        for di in DELTA_ORDER:
            delta, sz = DELTAS[di]
            if sz > c0:
                ps.append((di, delta, min(CCW, sz - c0)))
        side_w = sum(w for _, _, w in ps)
        for side in range(2):
            p = off + side * side_w
            for di, _, w in ps:
                wcol[(cc, side, di)] = p
                p += w
        pieces.append(ps)
        off += 2 * side_w
        choff.append(off)
    assert off == 2 * WSIDE
    return pieces, wcol, choff


PIECES, WCOL, CHOFF = _piece_tables()

_PROG = None


def _build_program():
    nc = bacc.Bacc("TRN2", target_bir_lowering=False, debug=False, num_devices=8)
    f32 = mybir.dt.float32
    bf16 = mybir.dt.bfloat16

    ps_d = nc.dram_tensor("ps", [D, PS_COLS], bf16, kind="ExternalInput")
    pq_d = nc.dram_tensor("pq", [D, PQ_COLS], bf16, kind="ExternalInput")
    w_d = nc.dram_tensor("w", [D, 2 * WSIDE], bf16, kind="ExternalInput")
    bias_d = nc.dram_tensor("bias", [CCW, NCK], f32, kind="ExternalInput")
    out_d = nc.dram_tensor("out", [NCH, NPDEV], bf16, kind="ExternalOutput")

    with tile.TileContext(nc) as tc:
        with (
            tc.tile_pool(name="persist", bufs=1) as big,
            tc.tile_pool(name="cpool", bufs=2) as cpool,
            tc.tile_pool(name="sumpool", bufs=2) as sumpool,
            tc.tile_pool(name="treepool", bufs=2) as treepool,
            tc.tile_pool(name="outpool", bufs=2) as outpool,
            tc.tile_pool(name="convps", bufs=2, space="PSUM") as convps,
        ):
            w_sb = big.tile([128, 4 * 2 * WSIDE], bf16, tag="w")
            ps_sb = big.tile([128, 4 * PS_COLS], bf16, tag="ps")
            pq_sb = big.tile([128, 4 * PQ_COLS], bf16, tag="pq")
            bias_sb = big.tile([CCW, NCK], f32, tag="bias")
            warm_sb = big.tile([128, 512], bf16, tag="warm")

            # hoist the one-time LoadActFuncSet to t=0 (it precedes the first
            # Activation instruction on the Act queue and has no data deps)
            nc.gpsimd.memset(warm_sb[:], 0.0)
            nc.scalar.copy(warm_sb[0:1, 256:257], warm_sb[0:1, 0:1])

            # keep the PE busy during the input-DMA prologue so the clock
            # gate is warm (2.4 GHz) when the first conv matmul lands
            warm_ps = convps.tile([128, 256], f32, tag="warm")
            for _wi in range(NWARM):
                nc.tensor.matmul(
                    warm_ps[0:128, 0:256],
                    lhsT=warm_sb[:, 0:128],
                    rhs=warm_sb[:, 0:256],
                    start=True,
                    stop=True,
                )

            def wload(cc):
                wd = w_d[:].rearrange("(d p) c -> p d c", p=128)
                ws = w_sb[:].rearrange("p (d c) -> p d c", c=2 * WSIDE)
                nc.sync.dma_start(
                    ws[:, :, CHOFF[cc] : CHOFF[cc + 1]],
                    wd[:, :, CHOFF[cc] : CHOFF[cc + 1]],
                )

            # prologue DMA order = first-use order: W for the first chunk,
            # then q positions (conv runs q first), bias (Act copy of the s
            # blocks needs it), then s positions
            wload(CHUNK_ORDER[0])
            nc.sync.dma_start(
                pq_sb[:].rearrange("p (d c) -> p d c", c=PQ_COLS),
                pq_d[:].rearrange("(d p) c -> p d c", p=128),
            )
            nc.sync.dma_start(bias_sb[:], bias_d[:])
            # s positions split at the S1/S2 block boundary so the S1 conv
            # (and with it the first pairwise add) starts ~1.2us earlier
            ps3 = ps_sb[:].rearrange("p (d c) -> p d c", c=PS_COLS)
            pd3 = ps_d[:].rearrange("(d p) c -> p d c", p=128)
            nc.sync.dma_start(
                ps3[:, :, 0 : SRA * ROWSTR], pd3[:, :, 0 : SRA * ROWSTR]
            )
            nc.sync.dma_start(
                ps3[:, :, SRA * ROWSTR : PS_COLS], pd3[:, :, SRA * ROWSTR : PS_COLS]
            )
            wload(CHUNK_ORDER[1])

            def conv(cc):
                """PE conv for channel chunk cc -> (cq, cs1, cs2) bf16 tiles."""
                outs = []
                for side, r0, nr in POS_CHUNKS:
                    src, cols = (ps_sb, PS_COLS) if side == 0 else (pq_sb, PQ_COLS)
                    psum = convps.tile([CCW, 403], f32, tag="conv")
                    mms = [
                        (d, di, delta, w)
                        for d in range(4)
                        for di, delta, w in PIECES[cc]
                    ]
                    for idx, (d, di, delta, w) in enumerate(mms):
                        lcol = d * 2 * WSIDE + WCOL[(cc, side, di)]
                        roff = d * cols + r0 * ROWSTR + 2 + delta
                        nc.tensor.matmul(
                            bass.AP(
                                psum[:].tensor,
                                psum[:].offset,
                                [[psum[:].ap[0][0], w], [L, nr], [1, L]],
                            ),
                            lhsT=w_sb[:, lcol : lcol + w],
                            rhs=bass.AP(
                                src[:].tensor,
                                src[:].offset + roff,
                                [[src[:].ap[0][0], 128], [ROWSTR, nr], [1, L]],
                            ),
                            start=(idx == 0),
                            stop=(idx == len(mms) - 1),
                        )
                    # PSUM -> SBUF bf16; bias folded into the s-side copies
                    ct = cpool.tile([CCW, nr * L], bf16, tag=f"c{side}{r0}")
                    if side == 0:
                        nc.scalar.add(
                            ct[0:CCW, 0 : nr * L],
                            psum[0:CCW, 0 : nr * L],
                            bias_sb[0:CCW, cc : cc + 1],
                        )
                    else:
                        nc.scalar.copy(ct[0:CCW, 0 : nr * L], psum[0:CCW, 0 : nr * L])
                    outs.append(ct)
                return outs

            def badd(eng, cs, csoff, cq, nr, jn, cqoff, sm, smoff):
                """sm[ch, i, t, l] = cs[ch,i,l] + cq[ch,t+off,l] (bf16 bcast)."""
                eng.tensor_tensor(
                    sm[0:CCW, smoff : smoff + nr * jn * L],
                    bass.AP(
                        cs[:].tensor,
                        cs[:].offset + csoff,
                        [[cs[:].ap[0][0], CCW], [L, nr], [0, jn], [1, L]],
                    ),
                    bass.AP(
                        cq[:].tensor,
                        cq[:].offset + cqoff,
                        [[cq[:].ap[0][0], CCW], [0, nr], [L, jn], [1, L]],
                    ),
                    op=mybir.AluOpType.add,
                )

            def tree(sm, smoff, np_, outt, p0, pfx):
                """DVE max over l on np_ pairs of sm: 31->16->8->4->2->1."""

                def tmax(dst, dw, src, sw, half, soff=0, dsoff=0):
                    nc.vector.tensor_tensor(
                        bass.AP(
                            dst[:].tensor,
                            dst[:].offset + dsoff,
                            [[dst[:].ap[0][0], CCW], [dw, np_], [1, dw]],
                        ),
                        bass.AP(
                            src[:].tensor,
                            src[:].offset + soff,
                            [[src[:].ap[0][0], CCW], [sw, np_], [1, dw]],
                        ),
                        bass.AP(
                            src[:].tensor,
                            src[:].offset + soff + half,
                            [[src[:].ap[0][0], CCW], [sw, np_], [1, dw]],
                        ),
                        op=mybir.AluOpType.max,
                    )

                t16 = treepool.tile([CCW, 248 * 16], bf16, tag=pfx + "16")
                t8 = treepool.tile([CCW, 248 * 8], bf16, tag=pfx + "8")
                t4 = treepool.tile([CCW, 248 * 4], bf16, tag=pfx + "4")
                t2 = treepool.tile([CCW, 248 * 2], bf16, tag=pfx + "2")
                tmax(t16, 16, sm, L, 15, soff=smoff)
                tmax(t8, 8, t16, 16, 8)
                tmax(t4, 4, t8, 8, 4)
                tmax(t2, 2, t4, 4, 2)
                tmax(outt, 1, t2, 2, 1, dsoff=p0)

            def finish(cc, outt):
                # relu (relu(max) == max(relu)) on the otherwise-idle Act
                # engine; by emission order it queues behind the next chunk's
                # PSUM copies and never stalls them
                nc.scalar.activation(
                    outt[0:CCW, 0:NPDEV],
                    outt[0:CCW, 0:NPDEV],
                    mybir.ActivationFunctionType.Relu,
                )
                nc.sync.dma_start(
                    out_d[cc * CCW : (cc + 1) * CCW, :], outt[0:CCW, 0:NPDEV]
                )

            prev = None  # (cc, outt, smP, np_pool) of the previous chunk
            for ci, cc in enumerate(CHUNK_ORDER):
                if ci + 2 < NCK:
                    wload(CHUNK_ORDER[ci + 2])
                cq, cs1, cs2 = conv(cc)
                npp = POOLR[ci] * JN
                outt = outpool.tile([CCW, NPDEV], bf16, tag="out")
                smP = sumpool.tile([CCW, NPA * L], bf16, tag="smP")
                smD = sumpool.tile([CCW, 248 * L], bf16, tag="smD")
                # Pool adds the first POOLR s-rows' pairs; DVE the rest
                badd(nc.gpsimd, cs1, 0, cq, POOLR[ci], JN, 0, smP, 0)
                badd(nc.vector, cs1, POOLR[ci] * L, cq, SRA - POOLR[ci], JN, 0,
                     smD, 0)
                badd(nc.vector, cs2, 0, cq, NROW - SRA, JNB, L, smD,
                     (NPA - npp) * L)
                tree(smD, 0, NPDEV - npp, outt, npp, "r")
                if prev is not None:
                    # previous chunk's Pool slice, treed at period end (its add
                    # finishes mid-period) so DVE rarely waits on Pool
                    pcc, poutt, psmP, pnpp = prev
                    tree(psmP, 0, pnpp, poutt, 0, "p")
                    finish(pcc, poutt)
                prev = (cc, outt, smP, npp)
            # drain the last chunk: its Pool add finished long before DVE gets
            # here, so tree + relu run engine-local and DMA out directly
            pcc, poutt, psmP, pnpp = prev
            tree(psmP, 0, pnpp, poutt, 0, "p")
            nc.vector.tensor_scalar_max(
                poutt[0:CCW, 0:NPDEV], poutt[0:CCW, 0:NPDEV], 0.0
            )
            nc.sync.dma_start(
                out_d[pcc * CCW : (pcc + 1) * CCW, :], poutt[0:CCW, 0:NPDEV]
            )

    nc.compile()
    return nc


def get_program():
    global _PROG
    if _PROG is None:
        _PROG = _build_program()
    return _PROG


def build_inputs(s, q, ws, bs):
    """Host-side shard prep. ws/bs: dicts k -> w(150, 1024, k) / b(150,).

    Returns in_maps. Core c handles episode c//2, q-row half c%2.
    """
    s = np.asarray(s, dtype=np.float32).reshape(B, NROW, L, D)
    q = np.asarray(q, dtype=np.float32).reshape(B, NQROW, L, D)

    # packed weights [D, 2*WSIDE] in device chunk-major piece order
    wfull = np.zeros((2, D, 5, NCH), dtype=np.float32)
    bias_dev = np.zeros(NCH, dtype=np.float32)
    for k in (2, 3, 4, 5):
        blk = ORD_OF_K[k] * 150
        bias_dev[blk : blk + 150] = bs[k]
        for di, (delta, sz) in enumerate(DELTAS):
            t = delta + PAD_OF_K[k]
            if not (0 <= t < k):
                continue
            assert blk + 150 <= sz
            wfull[0, :, di, blk : blk + 150] = ws[k][:, :D, t].T
            wfull[1, :, di, blk : blk + 150] = ws[k][:, D:, t].T
    cols = []
    for cc in range(NCK):
        c0 = cc * CCW
        for side in range(2):
            for di, _, w in PIECES[cc]:
                cols.append(wfull[side, :, di, c0 : c0 + w])
    wall = np.concatenate(cols, axis=1).astype(ml_dtypes.bfloat16)
    assert wall.shape == (D, 2 * WSIDE)

    bias_arr = np.ascontiguousarray(bias_dev.reshape(NCK, CCW).T)

    in_maps = []
    for core in range(8):
        b, jh = core // 2, core % 2
        iord, qord = _core_orders(jh)
        psa = np.zeros((D, PS_COLS), dtype=np.float32)
        pqa = np.zeros((D, PQ_COLS), dtype=np.float32)
        for r in range(NROW):
            psa[:, r * ROWSTR + 2 : r * ROWSTR + 2 + L] = s[b, iord[r]].T
        for t in range(JN):
            pqa[:, t * ROWSTR + 2 : t * ROWSTR + 2 + L] = q[b, qord[t]].T
        in_maps.append(
            {
                "ps": psa.astype(ml_dtypes.bfloat16),
                "pq": pqa.astype(ml_dtypes.bfloat16),
                "w": wall,
                "bias": bias_arr,
            }
        )
    return in_maps


def _core_orders(jh):
    """Device s-row / q-row orderings for a core's q-half jh.

    The 625 pairs of an episode split 313/312 between the two cores:
    block A = dev rows 0..13 x dev q 0..13, block B = dev rows 13..25 x
    dev q 1..13. The row orders below make those device blocks cover
    exactly the right real (i, q) rectangles on each core.
    """
    if jh == 0:
        iord = list(range(NROW))                # identity
        qord = [12] + list(range(12))           # A: q<=12, B(q 1..13): q<=11
    else:
        iord = [(d + 12) % NROW for d in range(NROW)]  # dev 0..13 -> i 12..24
        qord = list(range(12, NQROW))           # A: q>=12, B: q 13..24
    return iord, qord


def _dev_pair_map(jh):
    """dev pair index -> (real_i, real_q) arrays for a core."""
    iord, qord = _core_orders(jh)
    ri = np.empty(NPDEV, dtype=np.int64)
    rq = np.empty(NPDEV, dtype=np.int64)
    for i in range(SRA):
        for t in range(JN):
            p = i * JN + t
            ri[p], rq[p] = iord[i], qord[t]
    for i in range(SRA, NROW):
        for t in range(1, JN):
            p = NPA + (i - SRA) * JNB + (t - 1)
            ri[p], rq[p] = iord[i], qord[t]
    return ri, rq


# device channel -> original output channel maps
_S_IDX = np.array(
    [(3 - g) * 150 + u for g in range(4) for u in range(75)], dtype=np.int64
)
_Q_IDX = _S_IDX + 75


_PAIR_MAPS = [_dev_pair_map(0), _dev_pair_map(1)]


def assemble_outputs(core_outs):
    """core_outs: list of 8 arrays [NCH, NPDEV] -> (s_out, q_out)."""
    s_out = np.empty((B, NROW, NQROW, 300), dtype=np.float32)
    q_out = np.empty((B, NROW, NQROW, 300), dtype=np.float32)
    for core in range(8):
        b, jh = core // 2, core % 2
        ri, rq = _PAIR_MAPS[jh]
        arr = np.ascontiguousarray(np.asarray(core_outs[core]).astype(np.float32).T)
        s_out[b, ri, rq] = arr[:, _S_IDX]
        q_out[b, ri, rq] = arr[:, _Q_IDX]
    return s_out.reshape(-1, 300), q_out.reshape(-1, 300)


def kernel(s, q, w2, b2, w3, b3, w4, b4, w5, b5, B=4, N=5, K=5, Q=5, L=31):
    ws = {2: np.asarray(w2, np.float32), 3: np.asarray(w3, np.float32),
          4: np.asarray(w4, np.float32), 5: np.asarray(w5, np.float32)}
    bs = {2: np.asarray(b2, np.float32), 3: np.asarray(b3, np.float32),
          4: np.asarray(b4, np.float32), 5: np.asarray(b5, np.float32)}
    in_maps = build_inputs(s, q, ws, bs)
    nc = get_program()
    res = run_bass_kernel_spmd(nc, in_maps, list(range(8))).results
    return assemble_outputs([res[c]["out"] for c in range(8)])
